# revision 10
# baseline (speedup 1.0000x reference)
"""GQA kernel for 8 trn2 NeuronCores — zero-communication sequence sharding.

Layout: core c owns 256 query rows spread over 16 "stripes": for d in 1..16
it owns global rows [128*(d-1)+16c, 128*(d-1)+16c+16). Stripe d needs exactly
d key-tiles of 128, so every core has an identical causal instruction stream
(perfect balance, one SPMD program).

Pipeline per core (everything transposed so no PE transposes are needed):
  K^T[kd,k]  = wk^T x^T   (bf16 matmuls, fp32 PSUM)
  V[k,dv]    = x^T^T wv   (stored with a ones column per group -> softmax denom)
  Q^T[hd,q]  = wq^T x^T   (own 256 columns only)
  scores^T[k,q] per (head, ktile), exp on ACT (scale=1/8), causal diag mask,
  ctx^T[d,q] accumulated over ktiles with the ones row giving the denominator,
  normalize via reciprocal + partition-broadcast,
  out^T[e,q] = wo^T ctx^T in float32r (full fp32 precision, 1 cyc/row).

Host pre/post: transpose+cast inputs, scatter-gather the stripe rows.
"""

import sys

sys.path.insert(0, "/opt/trn_rl_repo")

import numpy as np
import ml_dtypes

BF16 = ml_dtypes.bfloat16

N = 2048          # seq len == model dim
NKV = 512         # kv proj width (8 groups * 64)
NH = 32           # heads
NG = 8            # kv groups
HD = 64           # head dim
NCORES = 8
MQ = 256          # query columns per core
NKT = 16          # key tiles of 128

LAST_EXEC_NS = None
LAST_RESULTS = None


def _build():
    import concourse.bacc as bacc
    import concourse.mybir as mybir
    import concourse.tile as tile

    f32 = mybir.dt.float32
    bf16 = mybir.dt.bfloat16
    f32r = mybir.dt.float32r
    EXP = mybir.ActivationFunctionType.Exp

    nc = bacc.Bacc("TRN2", target_bir_lowering=False, debug=False,
                   num_devices=NCORES)

    xT_d = nc.dram_tensor("xT", [N, N], bf16, kind="ExternalInput")
    xTo_d = nc.dram_tensor("xTown", [N, MQ], bf16, kind="ExternalInput")
    wq_d = nc.dram_tensor("wq", [N, N], bf16, kind="ExternalInput")
    wk_d = nc.dram_tensor("wk", [N, NKV], bf16, kind="ExternalInput")
    wv_d = nc.dram_tensor("wv", [N, NKV], bf16, kind="ExternalInput")
    wo_d = nc.dram_tensor("wo", [N, N], f32r, kind="ExternalInput")
    mask_d = nc.dram_tensor("mask", [128, 16], bf16, kind="ExternalInput")
    outT_d = nc.dram_tensor("outT", [N, MQ], f32, kind="ExternalOutput")

    with tile.TileContext(nc) as tc:
        with (
            tc.tile_pool(name="res", bufs=1) as res,
            tc.tile_pool(name="io", bufs=3) as io,
            tc.tile_pool(name="wstr", bufs=8) as wstr,
            tc.tile_pool(name="expp", bufs=4) as expp,
            tc.tile_pool(name="small", bufs=2) as small,
            tc.tile_pool(name="psA", bufs=1, space="PSUM") as psA,
            tc.tile_pool(name="psB", bufs=1, space="PSUM") as psB,
        ):
            zb = res.tile([128, 1], f32, tag="zb")
            nc.gpsimd.memset(zb[:], 0.0)
            mask_sb = res.tile([128, 16], bf16, tag="mask")
            nc.sync.dma_start(mask_sb[:], mask_d[:])

            # resident weights for the K/V projection + wq (all bf16)
            wk_sb = []
            wv_sb = []
            wq_sb = []
            for e in range(16):
                wkt = res.tile([128, NKV], bf16, tag=f"wk{e}")
                nc.sync.dma_start(wkt[:], wk_d[128 * e:128 * e + 128, :])
                wk_sb.append(wkt)
                wvt = res.tile([128, NKV], bf16, tag=f"wv{e}")
                nc.sync.dma_start(wvt[:], wv_d[128 * e:128 * e + 128, :])
                wv_sb.append(wvt)
                wqt = res.tile([128, N], bf16, tag=f"wq{e}")
                nc.sync.dma_start(wqt[:], wq_d[128 * e:128 * e + 128, :])
                wq_sb.append(wqt)

            # resident activations
            K_sb = [res.tile([128, N], bf16, tag=f"K{j}", name=f"K{j}") for j in range(4)]
            Vaug = res.tile([128, NKT * 520], bf16, tag="Vaug")
            Q_sb = res.tile([128, 16 * MQ], bf16, tag="Qsb")
            ctx_sb = res.tile([128, 16 * MQ], f32r, tag="ctx")

            # ones columns of Vaug (col 64 of each group slot)
            for kt in range(NKT):
                seg = Vaug[:, kt * 520:(kt + 1) * 520]
                seg3 = seg.rearrange("p (g c) -> p g c", c=65)
                nc.gpsimd.memset(seg3[:, :, 64:65], 1.0)

            # ---- phase 1: K^T and V projections over all 2048 key rows ----
            for kc in range(4):  # key chunks of 512
                kps = [psA.tile([128, 512], f32, tag=f"a{j}", name=f"kps{j}") for j in range(4)]
                vps = [psB.tile([128, 512], f32, tag=f"b{j}", name=f"vps{j}") for j in range(4)]
                for e in range(16):
                    xt = io.tile([128, 512], bf16, tag="xt")
                    nc.sync.dma_start(
                        xt[:], xT_d[128 * e:128 * e + 128, 512 * kc:512 * kc + 512])
                    for j in range(4):
                        nc.tensor.matmul(
                            kps[j][:], wk_sb[e][:, 128 * j:128 * j + 128], xt[:],
                            start=(e == 0), stop=(e == 15))
                    for j in range(4):
                        nc.tensor.matmul(
                            vps[j][:], xt[:, 128 * j:128 * j + 128], wv_sb[e][:],
                            start=(e == 0), stop=(e == 15))
                for j in range(4):
                    nc.scalar.copy(K_sb[j][:, 512 * kc:512 * kc + 512], kps[j][:])
                for j in range(4):
                    kt = 4 * kc + j
                    seg = Vaug[:, kt * 520:(kt + 1) * 520]
                    dst = seg.rearrange("p (g c) -> p g c", c=65)[:, :, 0:64]
                    src = vps[j].rearrange("p (g c) -> p g c", c=64)
                    nc.vector.tensor_copy(dst, src)

            # ---- phase 2: Q^T projection for own 256 query columns ----
            xto_sb = []
            for e in range(16):
                t = res.tile([128, MQ], bf16, tag=f"xto{e}")
                nc.sync.dma_start(t[:], xTo_d[128 * e:128 * e + 128, :])
                xto_sb.append(t)
            for i in range(16):
                qps = psA.tile([128, MQ], f32, tag=f"a{i % 2}", name="qps")
                for e in range(16):
                    nc.tensor.matmul(
                        qps[:], wq_sb[e][:, 128 * i:128 * i + 128], xto_sb[e][:],
                        start=(e == 0), stop=(e == 15))
                nc.scalar.copy(Q_sb[:, MQ * i:MQ * i + MQ], qps[:])

            # ---- phase 3: attention per head ----
            # Q_sb head slots are permuted (host-side wq column permutation)
            # so a head's partition parity matches its KV group's parity.
            EVEN = [h for h in range(NH) if (h // 4) % 2 == 0]
            ODD = [h for h in range(NH) if (h // 4) % 2 == 1]
            for qt in range(16):
                for qrow, h in ((0, EVEN[qt]), (64, ODD[qt])):
                    g = h // 4
                    krow = 64 * (g % 2)
                    ktile = g // 2
                    cps = psB.tile([65, MQ], f32, tag=f"b{2 + (2 * qt + qrow // 64) % 2}", name="cps")
                    for kt in range(NKT):
                        c0 = 16 * kt
                        sps = psB.tile([128, MQ], f32, tag=f"b{kt % 2}", name="sps")
                        nc.tensor.matmul(
                            sps[:, c0:MQ],
                            K_sb[ktile][krow:krow + 64, 128 * kt:128 * kt + 128],
                            Q_sb[qrow:qrow + 64, MQ * qt + c0:MQ * qt + MQ],
                            start=True, stop=True)
                        ex = expp.tile([128, MQ], bf16, tag="ex", name="ex")
                        nc.scalar.activation(ex[:, c0:MQ], sps[:, c0:MQ], EXP,
                                             bias=zb[:], scale=0.125)
                        nc.vector.tensor_mul(ex[:, c0:c0 + 16],
                                             ex[:, c0:c0 + 16], mask_sb[:])
                        nc.tensor.matmul(
                            cps[:, c0:MQ],
                            Vaug[:, kt * 520 + 65 * g:kt * 520 + 65 * g + 65],
                            ex[:, c0:MQ],
                            start=(kt == 0), stop=(kt == NKT - 1),
                            skip_group_check=True)
                    rcp = small.tile([1, MQ], f32, tag="rcp", name="rcp")
                    nc.vector.reciprocal(rcp[:], cps[64:65, :])
                    rbc = small.tile([64, MQ], f32, tag="rbc", name="rbc")
                    nc.gpsimd.partition_broadcast(rbc[:], rcp[:])
                    nc.vector.tensor_mul(
                        ctx_sb[qrow:qrow + 64, MQ * qt:MQ * qt + MQ],
                        cps[0:64, :], rbc[:])

            # ---- phase 4: output projection in float32r ----
            for et in range(16):
                ops = psA.tile([128, MQ], f32, tag=f"a{et % 2}", name="ops")
                for ht in range(16):
                    wot = wstr.tile([128, 128], f32r, tag="wot")
                    nc.sync.dma_start(
                        wot[:], wo_d[128 * ht:128 * ht + 128,
                                     128 * et:128 * et + 128])
                    nc.tensor.matmul(
                        ops[:], wot[:],
                        ctx_sb[:, MQ * ht:MQ * ht + MQ],
                        start=(ht == 0), stop=(ht == 15))
                osb = io.tile([128, MQ], f32, tag="osb")
                nc.vector.tensor_copy(osb[:], ops[:])
                nc.sync.dma_start(outT_d[128 * et:128 * et + 128, :], osb[:])

    nc.compile()
    return nc


def kernel(x, wq, wk, wv, wo, trace=False):
    global LAST_EXEC_NS, LAST_RESULTS
    from concourse.bass_utils import run_bass_kernel_spmd

    x = np.asarray(x)
    xT = np.ascontiguousarray(np.asarray(x[0], np.float32).T).astype(BF16)

    # head slot permutation: slot qt holds EVEN[qt] in rows 0-63 and
    # ODD[qt] in rows 64-127 (partition parity matches KV group parity)
    EVEN = [h for h in range(NH) if (h // 4) % 2 == 0]
    ODD = [h for h in range(NH) if (h // 4) % 2 == 1]
    hd_perm = np.empty(N, np.int64)
    for qt in range(16):
        hd_perm[128 * qt:128 * qt + 64] = EVEN[qt] * 64 + np.arange(64)
        hd_perm[128 * qt + 64:128 * qt + 128] = ODD[qt] * 64 + np.arange(64)

    wq_b = np.ascontiguousarray(
        np.asarray(wq, np.float32)[:, hd_perm]).astype(BF16)
    wk_b = np.asarray(wk, np.float32).astype(BF16)
    wv_b = np.asarray(wv, np.float32).astype(BF16)
    wo_f = np.ascontiguousarray(np.asarray(wo, np.float32)[hd_perm, :])

    in_maps = []
    for c in range(NCORES):
        cols = (128 * np.arange(16)[:, None] + 16 * c
                + np.arange(16)[None, :]).reshape(-1)
        xto = np.ascontiguousarray(xT[:, cols])
        mask = (np.arange(128)[:, None]
                <= (16 * c + np.arange(16))[None, :]).astype(BF16)
        in_maps.append({
            "xT": xT, "xTown": xto, "wq": wq_b, "wk": wk_b, "wv": wv_b,
            "wo": wo_f, "mask": mask,
        })

    nc = _build()
    res = run_bass_kernel_spmd(nc, in_maps, list(range(NCORES)), trace=trace)
    LAST_EXEC_NS = res.exec_time_ns
    LAST_RESULTS = res

    out = np.empty((2048, 2048), np.float32)
    out_r = out.reshape(16, NCORES, 16, 2048)
    for c in range(NCORES):
        oT = res.results[c]["outT"]  # [2048 e, 256 q]
        out_r[:, c, :, :] = oT.T.reshape(16, 16, 2048)
    return out[None].astype(np.float32)
